# revision 12
# baseline (speedup 1.0000x reference)
"""Trainium2 Bass kernel for nn_BGNN_MLP (bipartite 3-layer GNN).

Self-contained: kernel(**inputs) -> np.ndarray takes the full unsharded
inputs and returns the full [50000, 128] output, running on 8 NeuronCores
via run_bass_kernel_spmd.

Key algebraic restructuring: the Linear layers commute with the (linear)
segment-sum aggregations, so

  out = A2 A1 A0 X_v W012 + (A2 A1 deg0) c2^T + (A2 deg1) c1^T + deg2 c0^T

with W012 = W0 W1 W2, c2 = (W1 W2)^T b0, c1 = W2^T b1, c0 = b2, and
Ai the per-layer aggregation matrices (A0=A2=v2u, A1=u2v). The deg
vectors are pure graph structure, computed on host; W012/c* are computed
on device in a tiny preamble. The device mainline is then just three
sparse aggregations over 128-feature node tables:

  per layer: gather rows of the (AllGathered, bf16) source table via
  dma_gather into 128-slot tiles; one PE matmul per tile with a [128,96]
  0/1 selector as the stationary operand accumulates each tile into a
  96-destination "page" of a PSUM bank ([dest, feat] layout, fp32); the
  bank is copied (cast to bf16) into a staging slab, DMA'd out in a
  handful of large contiguous descriptors, and AllGathered in 4 chunks
  that overlap the remaining windows.

SPMD: one instruction stream for all 8 cores; all per-core variation is
carried by ExternalInput data (positions, gather indices, M selectors).
"""

import sys

if "/opt/trn_rl_repo" not in sys.path:
    sys.path.insert(0, "/opt/trn_rl_repo")

import numpy as np

NC = 8
D = 128
PAGE = 96          # destinations per PSUM page (MM output partitions)
PAGES = 4          # pages per PSUM bank (free-dim 512B each)
WINDOW = PAGE * PAGES  # 384 dests per window
SUBS = 4           # AllGather chunks per table

# ----------------------------------------------------------------------------
# host-side packing
# ----------------------------------------------------------------------------


def snake_positions(deg_local, nw):
    """Assign local dests to (window, page, slot) balancing rows per page.

    Returns pos[local_id] in [0, nw*WINDOW). Buckets = nw*PAGES pages,
    each gets <= PAGE dests, snake-dealt by descending degree.
    """
    loc = len(deg_local)
    nbuck = nw * PAGES
    order = np.argsort(-deg_local, kind="stable")
    pos = np.zeros(loc, np.int64)
    fill = np.zeros(nbuck, np.int64)
    b = 0
    direction = 1
    for i, d in enumerate(order):
        # snake over buckets
        tries = 0
        while fill[b] >= PAGE:
            b += direction
            if b == nbuck or b < 0:
                direction = -direction
                b += direction
            tries += 1
            assert tries <= 2 * nbuck
        pos[d] = b * PAGE + fill[b]
        fill[b] += 1
        b += direction
        if b == nbuck or b < 0:
            direction = -direction
            b += direction
    return pos


class DirPack:
    """One direction: dest packing + per-core tile/idx/M data."""

    def __init__(self, dest, src, n, loc, nw, src_row_of, pos):
        """src_row_of: [n] -> global row in the source table; pos: [n] ->
        local position of each dest within its core's table."""
        self.n, self.loc, self.nw = n, loc, nw
        self.R = nw * WINDOW
        self.pos = pos

        # edges sorted by dest
        order = np.argsort(dest, kind="stable")
        src_s = src[order]
        starts = np.concatenate([[0], np.cumsum(np.bincount(dest, minlength=n))])
        self.srow_s = src_row_of[src_s]
        self.starts = starts

    def build_tiles(self, half):
        """Compute per-(core, window, page, stream) tiles.

        Returns: T[w][pg][s] uniform tile counts, and per-core slot data.
        """
        n, loc, nw = self.n, self.loc, self.nw
        starts, srow_s = self.starts, self.srow_s
        # per-core, per-page, per-stream row lists (in position order)
        # rows_data[c][(w,pg,s)] = (list of (srcrow, slot_pos_in_page))
        core_rows = []
        cnt = np.zeros((NC, nw, PAGES, 2), np.int64)
        for c in range(NC):
            d0 = c * loc
            per_page = [[[ [], [] ] for _ in range(PAGES)] for _ in range(nw)]
            for li in range(loc):
                node = d0 + li
                q = self.pos[node]
                w, r = divmod(q, WINDOW)
                pg, p = divmod(r, PAGE)
                s0, e0 = starts[node], starts[node + 1]
                rows = srow_s[s0:e0]
                lo = rows[rows < half]
                hi = rows[rows >= half] - half
                if len(lo):
                    per_page[w][pg][0].append((p, lo))
                if len(hi):
                    per_page[w][pg][1].append((p, hi))
            for w in range(nw):
                for pg in range(PAGES):
                    for s in range(2):
                        tot = sum(len(r) for _, r in per_page[w][pg][s])
                        cnt[c, w, pg, s] = tot
            core_rows.append(per_page)
        # uniform tile counts
        T = np.ceil(cnt.max(axis=0) / 128).astype(np.int64)  # [nw, PAGES, 2]
        self.T = T
        self.core_rows = core_rows
        return T

    def emit_slabs(self, half):
        """Build idx + M slabs per core following the uniform schedule.

        Tile order (global): for w, for s in (lo, hi), for pg, for t.
        Gather calls: per (w, s): tiles of all pages consecutively, split
        into chunks of 8.
        """
        nw = self.nw
        T = self.T
        ntiles = int(T.sum())
        self.ntiles = ntiles
        # per-window tile counts per stream
        self.wtiles = [
            [int(T[w, :, s].sum()) for s in range(2)] for w in range(nw)
        ]
        idx_all = []
        m_all = []
        for c in range(NC):
            idx = np.zeros((ntiles, 128), np.int64)
            M = np.zeros((ntiles, 128, PAGE), np.float32)
            ti = 0
            per_page = self.core_rows[c]
            for w in range(nw):
                for s in range(2):
                    for pg in range(PAGES):
                        nt = int(T[w, pg, s])
                        if nt == 0:
                            continue
                        # flatten this page-stream's rows
                        prs = []
                        for p, rows in per_page[w][pg][s]:
                            for r in rows:
                                prs.append((r, p))
                        assert len(prs) <= nt * 128
                        for j, (r, p) in enumerate(prs):
                            t, sl = divmod(j, 128)
                            idx[ti + t, sl] = r
                            M[ti + t, sl, p] = 1.0
                        ti += nt
            assert ti == ntiles
            idx_all.append(idx)
            m_all.append(M)
        self.idx_all = idx_all
        self.m_all = m_all


def wrap_idx_calls(idx_tiles, wtiles):
    """Build the wrapped idx slab: per (window, stream) calls of <=8 tiles.

    idx_tiles: [ntiles, 128] in global tile order (w, s, pg, t).
    Returns [128, ncalls*64] int16 and the call schedule
    [(num_tiles, tile_base), ...].
    """
    schedule = []
    base = 0
    for w in range(len(wtiles)):
        for s in range(2):
            nt = wtiles[w][s]
            t0 = 0
            while t0 < nt:
                k = min(8, nt - t0)
                schedule.append((k, base + t0))
                t0 += k
            base += nt
    ncalls = len(schedule)
    out = np.zeros((16, ncalls * 64), np.int16)
    for ci, (k, tb) in enumerate(schedule):
        flat = idx_tiles[tb:tb + k].reshape(-1)  # k*128
        wrapped = flat.reshape(-1, 16).T  # [16, k*8]
        out[:, ci * 64: ci * 64 + k * 8] = wrapped
    return np.tile(out, (8, 1)), schedule


def bf16(x):
    """fp32 -> bf16 (round-to-nearest-even) stored as uint16 view."""
    x = np.asarray(x, np.float32)
    u = x.view(np.uint32)
    r = ((u >> 16) & 1) + 0x7FFF
    return ((u + r) >> 16).astype(np.uint16)


def prepare_host_data(inputs):
    eu = np.asarray(inputs["edge_u"]).astype(np.int64)
    ev = np.asarray(inputs["edge_v"]).astype(np.int64)
    X_v = np.asarray(inputs["X_v"], dtype=np.float32)
    n = int(np.asarray(inputs["X_u"]).shape[0])
    loc = n // NC
    nw = -(-loc // WINDOW)
    R = nw * WINDOW
    RG = R * NC
    half = RG // 2

    # AG chunk groups: SUBS groups of whole windows
    wgroups = np.array_split(np.arange(nw), SUBS)
    grows = np.array([len(wg) * WINDOW for wg in wgroups])      # local rows
    gcum = np.concatenate([[0], np.cumsum(grows)])              # local offsets
    ggcum = np.concatenate([[0], np.cumsum(grows * NC)])        # global offsets
    assert gcum[-1] == R and ggcum[-1] == RG

    def glob_rows(pos):
        """local position [n] -> global table row, group-chunked AG layout."""
        grp = np.searchsorted(gcum, pos, side="right") - 1
        cores = np.arange(n) // loc
        return ggcum[grp] + cores * grows[grp] + (pos - gcum[grp])

    # --- position maps for both dest spaces ---
    degU = np.bincount(eu, minlength=n)
    degV = np.bincount(ev, minlength=n)
    posU = np.zeros(n, np.int64)
    posV = np.zeros(n, np.int64)
    for c in range(NC):
        g = slice(c * loc, (c + 1) * loc)
        posU[g] = snake_positions(degU[g], nw)
        posV[g] = snake_positions(degV[g], nw)
    rowU = glob_rows(posU)
    rowV = glob_rows(posV)

    # packU: dests U (layers 0,2), sources V; packV: dests V (layer 1)
    packU = DirPack(eu, ev, n, loc, nw, rowV, posU)
    packU.build_tiles(half)
    packU.emit_slabs(half)

    packV = DirPack(ev, eu, n, loc, nw, rowU, posV)
    packV.build_tiles(half)
    packV.emit_slabs(half)

    # --- deg aggregation vectors (host, structure only) ---
    deg0 = degU.astype(np.float64)   # A0 * 1  (per U node)
    deg1 = degV.astype(np.float64)   # A1 * 1  (per V node)
    t_v = np.bincount(ev, weights=deg0[eu], minlength=n)   # A1 deg0 (per V)
    v2 = np.bincount(eu, weights=t_v[ev], minlength=n)     # A2 A1 deg0 (per U)
    v1 = np.bincount(eu, weights=deg1[ev], minlength=n)    # A2 deg1 (per U)
    v0 = degU.astype(np.float64)                           # deg2 (per U)

    # --- per-core external inputs ---
    W0 = np.asarray(inputs["W0"], np.float32)
    W1 = np.asarray(inputs["W1"], np.float32)
    W2 = np.asarray(inputs["W2"], np.float32)
    b0 = np.asarray(inputs["b0"], np.float32)
    b1 = np.asarray(inputs["b1"], np.float32)
    b2 = np.asarray(inputs["b2"], np.float32)

    bmat = np.zeros((128, 32), np.float32)
    bmat[:, 0] = b0
    bmat[:, 17] = b1
    cbase = np.zeros((16, 128), np.float32)
    cbase[2] = b2

    per_core = []
    for c in range(NC):
        g = np.arange(c * loc, (c + 1) * loc)
        xT = np.zeros((128, R), np.float32)
        xT[:, posV[g]] = X_v[g].T

        V3 = np.zeros((16, R), np.float32)
        V3[0, posU[g]] = v2[g]
        V3[1, posU[g]] = v1[g]
        V3[2, posU[g]] = v0[g]

        idxU_w, schedU = wrap_idx_calls(packU.idx_all[c], packU.wtiles)
        idxV_w, schedV = wrap_idx_calls(packV.idx_all[c], packV.wtiles)

        mU = packU.m_all[c].transpose(1, 0, 2).reshape(128, -1)  # [128, ntiles*96]
        mV = packV.m_all[c].transpose(1, 0, 2).reshape(128, -1)

        per_core.append({
            "xT": bf16(xT),
            "V3": V3,
            "idxU": idxU_w,
            "idxV": idxV_w,
            "mU": bf16(mU),
            "mV": bf16(mV),
            "W0T": W0.T.copy(),
            "W1T": W1.T.copy(),
            "W2": W2.copy(),
            "bmat": bmat,
            "cbase": cbase,
        })

    meta = {
        "n": n, "loc": loc, "nw": nw, "R": R, "RG": RG, "half": half,
        "wgroups": wgroups, "grows": grows, "gcum": gcum, "ggcum": ggcum,
        "schedU": schedU, "schedV": schedV,
        "packU": packU, "packV": packV,
        "posU": posU,
    }
    return meta, per_core


# ----------------------------------------------------------------------------
# walrus drain workaround: split multi-wait tail Drain into single-wait nops
# ----------------------------------------------------------------------------


def _patch_tile_drain():
    from concourse import tile
    if getattr(tile.TileContext, "_bgnn_drain_patched", False):
        return
    from concourse.vector_clock import ScopedClock

    def patched(self, tick_clock, wait_clock):
        nc = self.nc
        nops = [nc.sync.nop() for _ in range(31)]
        drain_inst = nc.sync.drain()
        wait_clock.add_sem_waits(
            drain_inst.ins, ScopedClock({None: tick_clock.global_clock})
        )
        si = drain_inst.ins.sync_info
        waits = list(si.on_wait) if si is not None else []
        if len(waits) > 1:
            assert len(waits) - 1 <= len(nops), len(waits)
            for i, w in enumerate(waits[:-1]):
                n = nops[i].ins
                nsi = n.sync_info
                if nsi is None:
                    n.sync_info = type(si)(on_wait=[w], on_update=[])
                else:
                    nsi.on_wait = list(nsi.on_wait) + [w]
            si.on_wait = waits[-1:]
        nc.all_engine_barrier()
        popped = nc._tile_sem_poison_stack.pop()
        assert popped is self._sem_poison
        nc.clear_and_free_semaphores(list(self.sems.allocated().values()))
        nc.all_engine_barrier()

    tile.TileContext._drain_and_barrier = patched
    tile.TileContext._bgnn_drain_patched = True


# ----------------------------------------------------------------------------
# device program
# ----------------------------------------------------------------------------


def build_program(meta, max_layers=3):
    import concourse.bass as bass
    import concourse.mybir as mybir
    from concourse import bacc, tile

    _patch_tile_drain()
    f32 = mybir.dt.float32
    bf = mybir.dt.bfloat16
    i16 = mybir.dt.int16

    nw, R, RG, half = meta["nw"], meta["R"], meta["RG"], meta["half"]
    wgroups, grows = meta["wgroups"], meta["grows"]
    gcum, ggcum = meta["gcum"], meta["ggcum"]
    packU, packV = meta["packU"], meta["packV"]
    schedU, schedV = meta["schedU"], meta["schedV"]
    ntU, ntV = packU.ntiles, packV.ntiles
    ncallU, ncallV = len(schedU), len(schedV)
    maxwtU = max(a + b for a, b in packU.wtiles)
    maxwtV = max(a + b for a, b in packV.wtiles)
    maxwt = max(maxwtU, maxwtV)

    nc = bacc.Bacc(num_swdge_queues=4)
    core_ids = list(range(NC))

    # I/O
    xT_d = nc.dram_tensor("xT", [128, R], bf, kind="ExternalInput")
    V3_d = nc.dram_tensor("V3", [16, R], f32, kind="ExternalInput")
    idxU_d = nc.dram_tensor("idxU", [128, ncallU * 64], i16, kind="ExternalInput")
    idxV_d = nc.dram_tensor("idxV", [128, ncallV * 64], i16, kind="ExternalInput")
    mU_d = nc.dram_tensor("mU", [128, ntU * PAGE], bf, kind="ExternalInput")
    mV_d = nc.dram_tensor("mV", [128, ntV * PAGE], bf, kind="ExternalInput")
    W0T_d = nc.dram_tensor("W0T", [128, 128], f32, kind="ExternalInput")
    W1T_d = nc.dram_tensor("W1T", [128, 128], f32, kind="ExternalInput")
    W2_d = nc.dram_tensor("W2", [128, 128], f32, kind="ExternalInput")
    bmat_d = nc.dram_tensor("bmat", [128, 32], f32, kind="ExternalInput")
    cbase_d = nc.dram_tensor("cbase", [16, 128], f32, kind="ExternalInput")
    out_d = nc.dram_tensor("outp", [R, 128], f32, kind="ExternalOutput")

    # internal DRAM: per-AG-group z chunk tensors for each produced table
    zP_d = [nc.dram_tensor(f"zP{c}", [int(grows[c]), 128], bf)
            for c in range(SUBS)]
    z0_d = [nc.dram_tensor(f"z0{c}", [int(grows[c]), 128], bf)
            for c in range(SUBS)]
    z1_d = [nc.dram_tensor(f"z1{c}", [int(grows[c]), 128], bf)
            for c in range(SUBS)]
    tabV_d = nc.dram_tensor("tabV", [RG, 128], bf, addr_space="Shared")
    tabU_d = nc.dram_tensor("tabU", [RG, 128], bf, addr_space="Shared")

    layers = [
        # (pack, sched, idx handle, m_d, src_tab, z chunk tensors, out tab)
        (packU, schedU, idxU_d, mU_d, tabV_d, z0_d, tabU_d),
        (packV, schedV, idxV_d, mV_d, tabU_d, z1_d, tabV_d),
        (packU, schedU, idxU_d, mU_d, tabV_d, None, None),
    ]

    with tile.TileContext(nc) as tc:
        with (
            tc.tile_pool(name="persist", bufs=1) as persist,
            tc.tile_pool(name="g", bufs=8) as g_pool,
            tc.tile_pool(name="mslab", bufs=2) as m_pool,
            tc.tile_pool(name="pagg", bufs=3, space="PSUM") as pagg_pool,
            tc.tile_pool(name="pdense", bufs=2, space="PSUM") as pdense_pool,
        ):
            xT_sb = persist.tile([128, R], bf, tag="xT")
            V3_sb = persist.tile([16, R], f32, tag="V3")
            idxU_sb = persist.tile([128, ncallU * 64], i16, tag="idxU")
            idxV_sb = persist.tile([128, ncallV * 64], i16, tag="idxV")
            w0t_sb = persist.tile([128, 128], f32, tag="w0t")
            w1t_sb = persist.tile([128, 128], f32, tag="w1t")
            w2_sb = persist.tile([128, 128], f32, tag="w2")
            bmat_sb = persist.tile([128, 32], f32, tag="bmat")
            cbase_sb = persist.tile([16, 128], f32, tag="cbase")
            w12_sb = persist.tile([128, 128], f32, tag="w12")
            w012_sb = persist.tile([128, 128], bf, tag="w012")
            C_sb = persist.tile([16, 128], f32, tag="C")
            stag = persist.tile([128, (R // PAGE) * 128], bf, tag="stag")
            stag32 = persist.tile([128, (R // PAGE) * 128], f32, tag="stag32")

            # preload
            nc.sync.dma_start(out=xT_sb[:], in_=xT_d[:])
            nc.sync.dma_start(out=V3_sb[:], in_=V3_d[:])
            nc.sync.dma_start(out=idxU_sb[:], in_=idxU_d[:])
            nc.sync.dma_start(out=idxV_sb[:], in_=idxV_d[:])
            nc.sync.dma_start(out=w0t_sb[:], in_=W0T_d[:])
            nc.sync.dma_start(out=w1t_sb[:], in_=W1T_d[:])
            nc.sync.dma_start(out=w2_sb[:], in_=W2_d[:])
            nc.sync.dma_start(out=bmat_sb[:], in_=bmat_d[:])
            nc.sync.dma_start(out=cbase_sb[:], in_=cbase_d[:])

            # ---- preamble: W12 = W1@W2; W012 = W0@W12; C rows ----
            p1 = pdense_pool.tile([128, 128], f32, tag="pw")
            nc.tensor.matmul(p1[:], lhsT=w1t_sb[:], rhs=w2_sb[:],
                             start=True, stop=True)
            nc.vector.tensor_copy(w12_sb[:], p1[:])
            p2 = pdense_pool.tile([128, 128], f32, tag="pw")
            nc.tensor.matmul(p2[:], lhsT=w0t_sb[:], rhs=w12_sb[:],
                             start=True, stop=True)
            nc.vector.tensor_copy(w012_sb[:], p2[:])
            pc = pdense_pool.tile([16, 128], f32, tag="pc")
            nc.tensor.matmul(pc[:], lhsT=bmat_sb[:, 0:16], rhs=w12_sb[:],
                             start=True, stop=False, skip_group_check=True)
            nc.tensor.matmul(pc[:], lhsT=bmat_sb[:, 16:32], rhs=w2_sb[:],
                             start=False, stop=True, skip_group_check=True)
            nc.vector.scalar_tensor_tensor(
                C_sb[:], cbase_sb[:], 0.0, pc[:],
                op0=mybir.AluOpType.add, op1=mybir.AluOpType.add)

            # ---- dense: zP = (X W012) rows at V positions ----
            nchunk = R // PAGE
            for k in range(nchunk):
                pz = pdense_pool.tile([128, 128], f32, tag="pz")
                nc.tensor.matmul(
                    pz[0:PAGE, :],
                    lhsT=xT_sb[:, k * PAGE:(k + 1) * PAGE],
                    rhs=w012_sb[:],
                    start=True, stop=True,
                )
                nc.vector.tensor_copy(
                    stag[0:PAGE, k * 128:(k + 1) * 128], pz[0:PAGE, :])
            # z-write + AG chunks (group tensors)
            for sc in range(SUBS):
                k0 = int(gcum[sc]) // PAGE
                k1 = int(gcum[sc + 1]) // PAGE
                dst = zP_d[sc].rearrange("(k p) f -> p k f", p=PAGE)
                src = stag[0:PAGE, k0 * 128:k1 * 128].rearrange(
                    "p (k f) -> p k f", f=128)
                nc.sync.dma_start(out=dst, in_=src)
                nc.gpsimd.collective_compute(
                    "AllGather", mybir.AluOpType.bypass,
                    replica_groups=[core_ids],
                    ins=[zP_d[sc][:]],
                    outs=[tabV_d[int(ggcum[sc]):int(ggcum[sc + 1]), :]],
                )

            gather_regs = {}
            call_no = [0]

            for li in range(max_layers):
                pack, sched, idx_sb_d, m_d, src_tab, z_d, out_tab = layers[li]
                idx_sb = idxU_sb if idx_sb_d is idxU_d else idxV_sb
                T = pack.T
                is_last = (li == 2)
                # call index base per (window, stream)
                ci = 0
                tile_base = 0

                for w in range(nw):
                    wt_lo, wt_hi = pack.wtiles[w]
                    wt = wt_lo + wt_hi
                    # gather calls for this window (lo then hi)
                    gtiles = []  # list of (gbuf, local offset) per tile
                    for s in range(2):
                        nt_s = pack.wtiles[w][s]
                        t0 = 0
                        while t0 < nt_s:
                            k, tb = sched[ci]
                            assert tb == tile_base + t0
                            gbuf = g_pool.tile([128, 8 * 128], bf, tag="g",
                                               name=f"g{li}")
                            nidx = k * 128
                            if nidx not in gather_regs:
                                gather_regs[nidx] = nc.gpsimd.to_reg(nidx)
                            src_ap = (src_tab[0:half, :] if s == 0
                                      else src_tab[half:RG, :])
                            nc.gpsimd.dma_gather(
                                gbuf[:, 0:k * 128].rearrange(
                                    "p (t e) -> p t e", e=128),
                                src_ap,
                                idx_sb[:, ci * 64:ci * 64 + k * 8],
                                num_idxs=nidx,
                                num_idxs_reg=gather_regs[nidx],
                                elem_size=128,
                                queue_num=call_no[0] % 4,
                            )
                            call_no[0] += 1
                            for j in range(k):
                                gtiles.append((gbuf, j))
                            ci += 1
                            t0 += k
                        tile_base += nt_s

                    # M slab for this window
                    mslab = m_pool.tile([128, maxwt * PAGE], bf, tag="m")
                    mbase = tile_base - wt
                    nc.sync.dma_start(
                        out=mslab[:, 0:wt * PAGE],
                        in_=m_d[:, mbase * PAGE:(mbase + wt) * PAGE])

                    # PSUM bank for this window
                    pw = pagg_pool.tile([128, 512], f32, tag="pagg")
                    if is_last:
                        for pg in range(PAGES):
                            nc.tensor.matmul(
                                pw[0:PAGE, pg * 128:(pg + 1) * 128],
                                lhsT=V3_sb[:, w * WINDOW + pg * PAGE:
                                           w * WINDOW + (pg + 1) * PAGE],
                                rhs=C_sb[:],
                                start=True, stop=False, skip_group_check=True,
                            )
                    # MMs: tile order within window = (s, pg, t);
                    # PSUM region = page
                    started = [is_last] * PAGES
                    ti_w = 0
                    # per-page remaining MM counts to set stop flags
                    mm_left = [int(T[w, pg, 0] + T[w, pg, 1])
                               for pg in range(PAGES)]
                    for s in range(2):
                        for pg in range(PAGES):
                            for t in range(int(T[w, pg, s])):
                                gbuf, j = gtiles[ti_w]
                                mm_left[pg] -= 1
                                nc.tensor.matmul(
                                    pw[0:PAGE, pg * 128:(pg + 1) * 128],
                                    lhsT=mslab[:, ti_w * PAGE:(ti_w + 1) * PAGE],
                                    rhs=gbuf[:, j * 128:(j + 1) * 128],
                                    start=not started[pg],
                                    stop=(mm_left[pg] == 0),
                                    skip_group_check=True,
                                )
                                started[pg] = True
                                ti_w += 1
                    assert ti_w == wt

                    # copy bank -> staging
                    if is_last:
                        nc.vector.tensor_copy(
                            stag32[0:PAGE, w * 512:(w + 1) * 512],
                            pw[0:PAGE, :])
                    else:
                        nc.vector.tensor_copy(
                            stag[0:PAGE, w * 512:(w + 1) * 512],
                            pw[0:PAGE, :])

                    # z-write + sub-AG at group boundaries
                    if not is_last:
                        for sc in range(SUBS):
                            if w == wgroups[sc][-1]:
                                wlo = int(wgroups[sc][0])
                                k0, k1 = wlo * PAGES, (w + 1) * PAGES
                                dst = z_d[sc].rearrange(
                                    "(k p) f -> p k f", p=PAGE)
                                src = stag[0:PAGE, k0 * 128:k1 * 128].rearrange(
                                    "p (k f) -> p k f", f=128)
                                nc.sync.dma_start(out=dst, in_=src)
                                nc.gpsimd.collective_compute(
                                    "AllGather", mybir.AluOpType.bypass,
                                    replica_groups=[core_ids],
                                    ins=[z_d[sc][:]],
                                    outs=[out_tab[int(ggcum[sc]):
                                                  int(ggcum[sc + 1]), :]],
                                )

            # final output write
            if max_layers == 3:
                dst = out_d.rearrange("(k p) f -> p k f", p=PAGE)
                src = stag32[0:PAGE, :].rearrange("p (k f) -> p k f", f=128)
                nc.sync.dma_start(out=dst, in_=src)

    nc.compile()
    return nc


# ----------------------------------------------------------------------------
# public entry point
# ----------------------------------------------------------------------------


def kernel(**inputs) -> np.ndarray:
    from concourse.bass_utils import run_bass_kernel_spmd

    meta, per_core = prepare_host_data(inputs)
    nc = build_program(meta)
    res = run_bass_kernel_spmd(nc, per_core, list(range(NC)))

    n, loc = meta["n"], meta["loc"]
    posU = meta["posU"]
    out = np.zeros((n, 128), np.float32)
    for c in range(NC):
        g = np.arange(c * loc, (c + 1) * loc)
        out[g] = res.results[c]["outp"][posU[g]]
    return out


if __name__ == "__main__":
    data = dict(np.load("/root/problem/inputs_cache.npz"))
    got = kernel(**data)
    np.save("/root/problem/kernel_out.npy", got)
    print("kernel done", got.shape)
